# revision 23
# baseline (speedup 1.0000x reference)
"""Trainium2 Bass kernel for nn_Attention_2 (8-head attention with positional bias).

Sharding: one head per NeuronCore (8 heads / 8 cores), data-parallel over the
full batch within each core.  Each core computes its head's projections,
attention (unnormalized softmax via exp(sim)*exp(bias) with the denominator
accumulated through a ones-column in V), and its partial contribution to the
output projection.  The host sums the 8 partial outputs.

Device layout notes:
- q/k/v are sent transposed ([d, n] per batch), bf16, and pre-tiled so every
  DMA is a single contiguous block.
- qh/kh are produced partition-replicated x2 (weights replicated on the host),
  so the K=32 sim matmuls run pairwise-concurrent in disjoint PE row groups.
- exp(pos_bias) is precomputed on the host (exp(sim+bias) = exp(sim)*exp(bias)),
  transposed to [j, i], tiled to match the sim PSUM layout, and sent in bf16.
- All matmul accumulation and the softmax denominator/normalization stay fp32;
  bf16 is used for matmul operands and the post-exp elementwise path.
"""

import sys

sys.path.insert(0, "/opt/trn_rl_repo")

import numpy as np
import ml_dtypes
from contextlib import ExitStack

import concourse.bass as bass  # noqa: F401
import concourse.tile as tile
from concourse import bacc, mybir
from concourse.bass_utils import run_bass_kernel_spmd

B, N, D, H, DH = 4, 2048, 256, 8, 32
SCALE = DH ** -0.5
NCORES = 8
IB = 512            # i-block (query columns per matmul)
NIB = N // IB       # 4
JC = 128            # j-chunk (key rows per partition tile)
NJC = N // JC       # 16
JP = 2              # j-chunks packed per PE pass (row groups)
NJG = NJC // JP     # 8
F32 = mybir.dt.float32
BF16 = mybir.dt.bfloat16
AF = mybir.ActivationFunctionType

QK_BF16 = True      # q/k/v inputs, projection weights, qh/kh in bf16


def build_kernel(nc, qT, kT, vT, wq, wk, wv, wo, eb, out, reps=1, var="base"):
    with tile.TileContext(nc) as tc:
        if reps == 1:
            _emit_body(nc, tc, qT, kT, vT, wq, wk, wv, wo, eb, out, var)
        else:
            with tc.For_i(0, reps, 1):
                _emit_body(nc, tc, qT, kT, vT, wq, wk, wv, wo, eb, out, var)


def _emit_body(nc, tc, qT, kT, vT, wq, wk, wv, wo, eb, out, var="base"):
    QKD = BF16 if QK_BF16 else F32
    with ExitStack() as ctx:
        consts = ctx.enter_context(tc.tile_pool(name="consts", bufs=1))
        persist = ctx.enter_context(tc.tile_pool(name="persist", bufs=1))
        qkv_pool = ctx.enter_context(tc.tile_pool(name="qkv", bufs=6))
        work = ctx.enter_context(tc.tile_pool(name="work", bufs=12 if var == "wb12" else 8))
        outp = ctx.enter_context(tc.tile_pool(name="outp", bufs=4))
        psS = ctx.enter_context(tc.tile_pool(name="psS", bufs=3 if var == "bout" else 2, space="PSUM"))
        psO = ctx.enter_context(tc.tile_pool(name="psO", bufs=2 if var == "bout" else 4, space="PSUM"))

        wq_sb = consts.tile([128, 2, JP * DH], QKD)
        nc.sync.dma_start(wq_sb[:], wq[:, :, :])
        wk_sb = consts.tile([128, 2, JP * DH], QKD)
        nc.sync.dma_start(wk_sb[:], wk[:, :, :])
        wv_sb = consts.tile([128, 2, DH], QKD)
        nc.sync.dma_start(wv_sb[:], wv[:, :, :])
        wo_sb = consts.tile([DH, D], F32)
        nc.sync.dma_start(wo_sb[:], wo[:, :])
        ident = consts.tile([128, 1], F32)
        nc.vector.memset(ident[:], 1.0)

        qh = [persist.tile([JP * DH, N], QKD, name=f"qh{b}") for b in range(B)]
        kh = [persist.tile([JP * DH, N], QKD, name=f"kh{b}") for b in range(B)]
        vh = [persist.tile([128, NJC, DH + 1], BF16, name=f"vh{b}") for b in range(B)]

        # ---- projections (one [*, ib] chunk of all batches) ----
        def emit_proj_chunk(ib):
            qdma = nc.scalar if var == "qsc" else nc.sync
            isl = slice(ib * IB, (ib + 1) * IB)
            for b in range(B):
                qt = qkv_pool.tile([128, 2, IB], QKD, name="qt")
                qdma.dma_start(qt[:], qT[b, ib])
                psq = psS.tile([JP * DH, IB], F32, name="psq", tag="s")
                nc.tensor.matmul(psq[:], wq_sb[:, 0, :], qt[:, 0, :], start=True, stop=False)
                nc.tensor.matmul(psq[:], wq_sb[:, 1, :], qt[:, 1, :], start=False, stop=True)
                nc.vector.tensor_copy(qh[b][:, isl], psq[:])

                kt = qkv_pool.tile([128, 2, IB], QKD, name="kt")
                qdma.dma_start(kt[:], kT[b, ib])
                psk = psS.tile([JP * DH, IB], F32, name="psk", tag="s")
                nc.tensor.matmul(psk[:], wk_sb[:, 0, :], kt[:, 0, :], start=True, stop=False)
                nc.tensor.matmul(psk[:], wk_sb[:, 1, :], kt[:, 1, :], start=False, stop=True)
                nc.vector.tensor_copy(kh[b][:, isl], psk[:])

                vt = qkv_pool.tile([128, 2, IB], QKD, name="vt")
                qdma.dma_start(vt[:], vT[b, ib])
                psv = psS.tile([128, IB // JC, DH], F32, name="psv", tag="s")
                for jl in range(IB // JC):
                    jsl = slice(jl * JC, (jl + 1) * JC)
                    nc.tensor.matmul(psv[:, jl, :], vt[:, 0, jsl], wv_sb[:, 0, :],
                                     start=True, stop=False, skip_group_check=True)
                    nc.tensor.matmul(psv[:, jl, :], vt[:, 1, jsl], wv_sb[:, 1, :],
                                     start=False, stop=True, skip_group_check=True)
                nc.scalar.copy(vh[b][:, ib * (IB // JC) : (ib + 1) * (IB // JC), 0:DH], psv[:])

        for b in range(B):
            nc.vector.memset(vh[b][:, :, DH : DH + 1], 1.0)

        # ---- attention + output projection ----
        def emit_attnv(unit):
            b, jg, wt = unit
            for t in range(JP):
                jc = jg * JP + t
                nc.tensor.matmul(
                    po[b][:], vh[b][:, jc, :], wt[:, t, :],
                    start=(jc == 0), stop=(jc == NJC - 1),
                )

        state = {"pending": None, "hpend": []}

        def emit_jg(ib, jg):
            # software-pipeline attn@v one unit late so the in-order PE
            # stream never waits on the DVE multiply
            isl = slice(ib * IB, (ib + 1) * IB)
            ebt = work.tile([128, JP, IB], BF16, name="ebt")
            nc.sync.dma_start(ebt[:], eb[ib, jg])
            for b in range(B):
                ps = psS.tile([128, JP, IB], F32, name="ps", tag="s")
                for t in range(JP):
                    jc = jg * JP + t
                    psl = slice(32 * t, 32 * (t + 1))
                    nc.tensor.matmul(
                        ps[:, t, :],
                        kh[b][psl, jc * JC : (jc + 1) * JC],
                        qh[b][psl, isl],
                        start=True, stop=True,
                    )
                es = work.tile([128, JP, IB], BF16, name="es")
                nc.scalar.activation(es[:], ps[:], AF.Exp)
                wt = work.tile([128, JP, IB], BF16, name="wt")
                meng = nc.gpsimd if (var == "pool" and b == 3) else nc.vector
                meng.tensor_mul(wt[:], es[:], ebt[:])
                if state["pending"] is not None:
                    emit_attnv(state["pending"])
                state["pending"] = (b, jg, wt)

        def emit_unit(ib, b, jg, ebt):
            isl = slice(ib * IB, (ib + 1) * IB)
            ps = psS.tile([128, JP, IB], F32, name="ps", tag="s")
            for t in range(JP):
                jc = jg * JP + t
                psl = slice(32 * t, 32 * (t + 1))
                nc.tensor.matmul(
                    ps[:, t, :],
                    kh[b][psl, jc * JC : (jc + 1) * JC],
                    qh[b][psl, isl],
                    start=True, stop=True,
                )
            es = work.tile([128, JP, IB], BF16, name="es")
            nc.scalar.activation(es[:], ps[:], AF.Exp)
            wt = work.tile([128, JP, IB], BF16, name="wt")
            nc.vector.tensor_mul(wt[:], es[:], ebt[:])
            if state["pending"] is not None:
                emit_attnv(state["pending"])
            state["pending"] = (b, jg, wt)

        def emit_attnv_half(h):
            b, jc, wt_t = h
            nc.tensor.matmul(
                po[b][:], vh[b][:, jc, :], wt_t[:],
                start=(jc == 0), stop=(jc == NJC - 1),
            )

        def emit_jg_esplit(ib, jg):
            isl = slice(ib * IB, (ib + 1) * IB)
            ebt = work.tile([128, JP, IB], BF16, name="ebt")
            nc.sync.dma_start(ebt[:], eb[ib, jg])
            for b in range(B):
                ps = psS.tile([128, JP, IB], F32, name="ps", tag="s")
                for t in range(JP):
                    jc = jg * JP + t
                    psl = slice(32 * t, 32 * (t + 1))
                    nc.tensor.matmul(
                        ps[:, t, :],
                        kh[b][psl, jc * JC : (jc + 1) * JC],
                        qh[b][psl, isl],
                        start=True, stop=True,
                    )
                for t in range(JP):
                    jc = jg * JP + t
                    es_t = work.tile([128, IB], BF16, name="esh")
                    nc.scalar.activation(es_t[:], ps[:, t, :], AF.Exp)
                    wt_t = work.tile([128, IB], BF16, name="wth")
                    nc.vector.tensor_mul(wt_t[:], es_t[:], ebt[:, t, :])
                    if state["hpend"]:
                        emit_attnv_half(state["hpend"].pop(0))
                    state["hpend"].append((b, jc, wt_t))

        def emit_tail_b(ib, b, po_b):
            ot = outp.tile([DH + 1, IB], F32, name="ot")
            nc.vector.tensor_copy(ot[:], po_b[:])
            zt = psO.tile([128, NIB], F32, name="zt", tag="po")
            for t_ in range(IB // JC):
                nc.tensor.matmul(
                    zt[:, t_ : t_ + 1],
                    ot[DH : DH + 1, t_ * JC : (t_ + 1) * JC],
                    ident[DH : DH + 1, :],
                    is_transpose=True,
                    skip_group_check=True,
                )
            rt = outp.tile([128, NIB], F32, name="rt")
            nc.vector.reciprocal(rt[:], zt[:])
            for t_ in range(IB // JC):
                pw = psO.tile([128, D], F32, name="pw", tag="po")
                nc.tensor.matmul(pw[:], ot[0:DH, t_ * JC : (t_ + 1) * JC], wo_sb[:], start=True, stop=True)
                ft = outp.tile([128, D], F32, name="ft")
                nc.vector.tensor_scalar_mul(ft[:], pw[:], rt[:, t_ : t_ + 1])
                r0 = ib * IB + t_ * JC
                nc.scalar.dma_start(out[b, r0 : r0 + JC, :], ft[:])

        if var == "bout":
            # batch-outer attention: one attn@v accumulator live at a time
            # (2 po banks double-buffered), 3 sim-psum slots, eb tiles cached
            # in SBUF across the 4 batch sweeps of each i-block
            for ib in range(NIB):
                ebts = [None] * NJG

                def get_ebt(jg, _ebts=ebts, _ib=ib):
                    if _ebts[jg] is None:
                        t = work.tile([128, JP, IB], BF16, name="ebt", tag="ebtc", bufs=16)
                        nc.sync.dma_start(t[:], eb[_ib, jg])
                        _ebts[jg] = t
                    return _ebts[jg]

                for b in range(B):
                    po_b = psO.tile([DH + 1, IB], F32, name="pob", tag="po")
                    po = [po_b] * B  # emit_attnv indexes po[b]
                    if ib == 0 and b == 0:
                        for ibk in range(NIB):
                            emit_proj_chunk(ibk)
                            emit_unit(0, 0, 2 * ibk, get_ebt(2 * ibk))
                            emit_unit(0, 0, 2 * ibk + 1, get_ebt(2 * ibk + 1))
                    else:
                        for jg in range(NJG):
                            emit_unit(ib, b, jg, get_ebt(jg))
                    emit_attnv(state["pending"])
                    state["pending"] = None
                    emit_tail_b(ib, b, po_b)
            return

        for ib in range(NIB):
            po = [psO.tile([DH + 1, IB], F32, name=f"po{b}", tag="po") for b in range(B)]
            if ib == 0:
                # interleave projection chunks with the first i-block's sweep:
                # attention(ib=0, jg) only needs k/v projection chunk jg//2
                ejg = emit_jg_esplit if var == "esplit" else emit_jg
                for ibk in range(NIB):
                    emit_proj_chunk(ibk)
                    ejg(0, 2 * ibk)
                    ejg(0, 2 * ibk + 1)
            else:
                for jg in range(NJG):
                    (emit_jg_esplit if var == "esplit" else emit_jg)(ib, jg)
            if state["pending"] is not None:
                emit_attnv(state["pending"])
            state["pending"] = None
            while state["hpend"]:
                emit_attnv_half(state["hpend"].pop(0))
            for b in range(B):
                ot = outp.tile([DH + 1, IB], F32, name="ot")
                if var == "otact":
                    nc.scalar.copy(ot[:], po[b][:])
                else:
                    nc.vector.tensor_copy(ot[:], po[b][:])
                zt = psO.tile([128, NIB], F32, name="zt", tag="po")
                for s in range(IB // JC):
                    nc.tensor.matmul(
                        zt[:, s : s + 1],
                        ot[DH : DH + 1, s * JC : (s + 1) * JC],
                        ident[DH : DH + 1, :],
                        is_transpose=True,
                        skip_group_check=True,
                    )
                rt = outp.tile([128, NIB], F32, name="rt")
                nc.vector.reciprocal(rt[:], zt[:])
                for s in range(IB // JC):
                    pw = psO.tile([128, D], F32, name="pw", tag="po")
                    nc.tensor.matmul(pw[:], ot[0:DH, s * JC : (s + 1) * JC], wo_sb[:], start=True, stop=True)
                    ft = outp.tile([128, D], F32, name="ft")
                    if var == "ftact" and s % 2 == 0:
                        nc.scalar.mul(ft[:], pw[:], rt[:, s : s + 1])
                    else:
                        nc.vector.tensor_scalar_mul(ft[:], pw[:], rt[:, s : s + 1])
                    r0 = ib * IB + s * JC
                    sdma = nc.sync if (var == "st2" and s % 2 == 0) else nc.scalar
                    sdma.dma_start(out[b, r0 : r0 + JC, :], ft[:])


_CACHE = {}


def _get_nc(reps=1, var="base"):
    key = ("nc", reps, var)
    if key not in _CACHE:
        QKD = BF16 if QK_BF16 else F32
        nc = bacc.Bacc("TRN2", target_bir_lowering=False, debug=False, num_devices=NCORES)
        qT = nc.dram_tensor("qT", [B, NIB, 128, 2, IB], QKD, kind="ExternalInput")
        kT = nc.dram_tensor("kT", [B, NIB, 128, 2, IB], QKD, kind="ExternalInput")
        vT = nc.dram_tensor("vT", [B, NIB, 128, 2, IB], QKD, kind="ExternalInput")
        wq = nc.dram_tensor("wq", [128, 2, JP * DH], QKD, kind="ExternalInput")
        wk = nc.dram_tensor("wk", [128, 2, JP * DH], QKD, kind="ExternalInput")
        wv = nc.dram_tensor("wv", [128, 2, DH], QKD, kind="ExternalInput")
        wo = nc.dram_tensor("wo", [DH, D], F32, kind="ExternalInput")
        eb = nc.dram_tensor("eb", [NIB, NJG, 128, JP, IB], BF16, kind="ExternalInput")
        out = nc.dram_tensor("out", [B, N, D], F32, kind="ExternalOutput")
        build_kernel(
            nc,
            qT.ap(), kT.ap(), vT.ap(),
            wq.ap(), wk.ap(), wv.ap(), wo.ap(),
            eb.ap(), out.ap(),
            reps=reps, var=var,
        )
        nc.compile()
        _CACHE[key] = nc
    return _CACHE[key]


def _np_dtype():
    return ml_dtypes.bfloat16 if QK_BF16 else np.float32


def _dn_layout(x):
    """[B, N, D] -> [B, NIB, 128, 2, IB]; tile (b, ib)[p, c, col] = x[b, ib*IB+col, c*128+p]."""
    t = x.reshape(B, NIB, IB, 2, 128)
    return np.ascontiguousarray(t.transpose(0, 1, 4, 3, 2).astype(_np_dtype()))


def _w_layout(w, rep):
    """[32, 256] (out, in) -> [128, 2, rep*32] transposed, M-replicated."""
    wt = np.ascontiguousarray(w.T)                       # [256, 32]
    wt = np.concatenate([wt] * rep, axis=1)              # [256, rep*32]
    return np.ascontiguousarray(
        wt.reshape(2, 128, rep * DH).transpose(1, 0, 2).astype(_np_dtype())
    )


def _eb_layout(pb_h):
    """[N, N] pos_bias head -> [NIB, NJG, 128, JP, IB] tiled exp-bias (bf16).
    tile (ib, jg)[p, t, col] = exp(pb_h[ib*IB+col, jg*(JP*128)+t*128+p])."""
    e = np.exp(pb_h)                                     # [i, j]
    x = e.reshape(NIB, IB, NJG, JP, 128)                 # [ib, col, jg, t, p]
    return np.ascontiguousarray(x.transpose(0, 2, 4, 3, 1).astype(ml_dtypes.bfloat16))


def make_in_maps(q, k, v, pos_bias, Wq, Wk, Wv, Wo):
    q = np.asarray(q, dtype=np.float32)
    k = np.asarray(k, dtype=np.float32)
    v = np.asarray(v, dtype=np.float32)
    pos_bias = np.asarray(pos_bias, dtype=np.float32)
    Wq = np.asarray(Wq, dtype=np.float32)
    Wk = np.asarray(Wk, dtype=np.float32)
    Wv = np.asarray(Wv, dtype=np.float32)
    Wo = np.asarray(Wo, dtype=np.float32)

    qT = _dn_layout(q)
    kT = _dn_layout(k)
    vT = _dn_layout(v)

    in_maps = []
    for h in range(NCORES):
        hs = slice(h * DH, (h + 1) * DH)
        in_maps.append({
            "qT": qT,
            "kT": kT,
            "vT": vT,
            "wq": _w_layout(SCALE * Wq[hs, :], JP),
            "wk": _w_layout(Wk[hs, :], JP),
            "wv": _w_layout(Wv[hs, :], 1),
            "wo": np.ascontiguousarray(Wo[:, hs].T),
            "eb": _eb_layout(pos_bias[h]),
        })
    return in_maps


def kernel(q, k, v, pos_bias, Wq, Wk, Wv, Wo):
    nc = _get_nc()
    in_maps = make_in_maps(q, k, v, pos_bias, Wq, Wk, Wv, Wo)
    res = run_bass_kernel_spmd(nc, in_maps, core_ids=list(range(NCORES)))
    acc = res.results[0]["out"].astype(np.float32)
    for c in range(1, NCORES):
        acc = acc + res.results[c]["out"]
    return acc


# revision 25
# speedup vs baseline: 1.0129x; 1.0129x over previous
"""Trainium2 Bass kernel for nn_Attention_2 (8-head attention with positional bias).

Sharding: one head per NeuronCore (8 heads / 8 cores), data-parallel over the
full batch within each core.  Each core computes its head's projections,
attention (unnormalized softmax via exp(sim)*exp(bias) with the denominator
accumulated through a ones-column in V), and its partial contribution to the
output projection.  The host sums the 8 partial outputs.

Device layout notes:
- q/k/v are sent transposed ([d, n] per batch), bf16, and pre-tiled so every
  DMA is a single contiguous block.
- qh/kh are produced partition-replicated x2 (weights replicated on the host),
  so the K=32 sim matmuls run pairwise-concurrent in disjoint PE row groups.
- exp(pos_bias) is precomputed on the host (exp(sim+bias) = exp(sim)*exp(bias)),
  transposed to [j, i], tiled to match the sim PSUM layout, and sent in bf16.
- All matmul accumulation and the softmax denominator/normalization stay fp32;
  bf16 is used for matmul operands and the post-exp elementwise path.
"""

import sys

sys.path.insert(0, "/opt/trn_rl_repo")

import numpy as np
import ml_dtypes
from contextlib import ExitStack

import concourse.bass as bass  # noqa: F401
import concourse.tile as tile
from concourse import bacc, mybir
from concourse.bass_utils import run_bass_kernel_spmd

B, N, D, H, DH = 4, 2048, 256, 8, 32
SCALE = DH ** -0.5
NCORES = 8
IB = 512            # i-block (query columns per matmul)
NIB = N // IB       # 4
JC = 128            # j-chunk (key rows per partition tile)
NJC = N // JC       # 16
JP = 2              # j-chunks packed per PE pass (row groups)
NJG = NJC // JP     # 8
F32 = mybir.dt.float32
BF16 = mybir.dt.bfloat16
AF = mybir.ActivationFunctionType

QK_BF16 = True      # q/k/v inputs, projection weights, qh/kh in bf16


def build_kernel(nc, qT, kT, vT, wq, wk, wv, wo, eb, out, reps=1, var="base"):
    with tile.TileContext(nc) as tc:
        if reps == 1:
            _emit_body(nc, tc, qT, kT, vT, wq, wk, wv, wo, eb, out, var)
        else:
            with tc.For_i(0, reps, 1):
                _emit_body(nc, tc, qT, kT, vT, wq, wk, wv, wo, eb, out, var)


def _emit_body(nc, tc, qT, kT, vT, wq, wk, wv, wo, eb, out, var="base"):
    QKD = BF16 if QK_BF16 else F32
    with ExitStack() as ctx:
        consts = ctx.enter_context(tc.tile_pool(name="consts", bufs=1))
        persist = ctx.enter_context(tc.tile_pool(name="persist", bufs=1))
        qkv_pool = ctx.enter_context(tc.tile_pool(name="qkv", bufs=6))
        work = ctx.enter_context(tc.tile_pool(name="work", bufs=12 if var == "wb12" else 8))
        outp = ctx.enter_context(tc.tile_pool(name="outp", bufs=4))
        psS = ctx.enter_context(tc.tile_pool(name="psS", bufs=3 if var == "bout" else 2, space="PSUM"))
        psO = ctx.enter_context(tc.tile_pool(name="psO", bufs=2 if var == "bout" else 4, space="PSUM"))

        wq_sb = consts.tile([128, 2, JP * DH], QKD)
        nc.sync.dma_start(wq_sb[:], wq[:, :, :])
        wk_sb = consts.tile([128, 2, JP * DH], QKD)
        nc.sync.dma_start(wk_sb[:], wk[:, :, :])
        wv_sb = consts.tile([128, 2, DH], QKD)
        nc.sync.dma_start(wv_sb[:], wv[:, :, :])
        wo_sb = consts.tile([DH, D], F32)
        nc.sync.dma_start(wo_sb[:], wo[:, :])
        ident = consts.tile([128, 1], F32)
        nc.vector.memset(ident[:], 1.0)

        qh = [persist.tile([JP * DH, N], QKD, name=f"qh{b}") for b in range(B)]
        kh = [persist.tile([JP * DH, N], QKD, name=f"kh{b}") for b in range(B)]
        vh = [persist.tile([128, NJC, DH + 1], BF16, name=f"vh{b}") for b in range(B)]

        # ---- projections (one [*, ib] chunk of all batches) ----
        def emit_proj_chunk(ib):
            qdma = nc.scalar if var == "qsc" else nc.sync
            isl = slice(ib * IB, (ib + 1) * IB)
            for b in range(B):
                qt = qkv_pool.tile([128, 2, IB], QKD, name="qt")
                qdma.dma_start(qt[:], qT[b, ib])
                psq = psS.tile([JP * DH, IB], F32, name="psq", tag="s")
                nc.tensor.matmul(psq[:], wq_sb[:, 0, :], qt[:, 0, :], start=True, stop=False)
                nc.tensor.matmul(psq[:], wq_sb[:, 1, :], qt[:, 1, :], start=False, stop=True)
                nc.vector.tensor_copy(qh[b][:, isl], psq[:])

                kt = qkv_pool.tile([128, 2, IB], QKD, name="kt")
                qdma.dma_start(kt[:], kT[b, ib])
                psk = psS.tile([JP * DH, IB], F32, name="psk", tag="s")
                nc.tensor.matmul(psk[:], wk_sb[:, 0, :], kt[:, 0, :], start=True, stop=False)
                nc.tensor.matmul(psk[:], wk_sb[:, 1, :], kt[:, 1, :], start=False, stop=True)
                nc.vector.tensor_copy(kh[b][:, isl], psk[:])

                vt = qkv_pool.tile([128, 2, IB], QKD, name="vt")
                qdma.dma_start(vt[:], vT[b, ib])
                psv = psS.tile([128, IB // JC, DH], F32, name="psv", tag="s")
                for jl in range(IB // JC):
                    jsl = slice(jl * JC, (jl + 1) * JC)
                    nc.tensor.matmul(psv[:, jl, :], vt[:, 0, jsl], wv_sb[:, 0, :],
                                     start=True, stop=False, skip_group_check=True)
                    nc.tensor.matmul(psv[:, jl, :], vt[:, 1, jsl], wv_sb[:, 1, :],
                                     start=False, stop=True, skip_group_check=True)
                nc.vector.tensor_copy(vh[b][:, ib * (IB // JC) : (ib + 1) * (IB // JC), 0:DH], psv[:])

        for b in range(B):
            nc.vector.memset(vh[b][:, :, DH : DH + 1], 1.0)

        # ---- attention + output projection ----
        def emit_attnv(unit):
            b, jg, wt = unit
            for t in range(JP):
                jc = jg * JP + t
                nc.tensor.matmul(
                    po[b][:], vh[b][:, jc, :], wt[:, t, :],
                    start=(jc == 0), stop=(jc == NJC - 1),
                )

        state = {"pending": None, "hpend": []}

        def emit_jg(ib, jg):
            # software-pipeline attn@v one unit late so the in-order PE
            # stream never waits on the DVE multiply
            isl = slice(ib * IB, (ib + 1) * IB)
            ebt = work.tile([128, JP, IB], BF16, name="ebt")
            nc.sync.dma_start(ebt[:], eb[ib, jg])
            for b in range(B):
                ps = psS.tile([128, JP, IB], F32, name="ps", tag="s")
                for t in range(JP):
                    jc = jg * JP + t
                    psl = slice(32 * t, 32 * (t + 1))
                    nc.tensor.matmul(
                        ps[:, t, :],
                        kh[b][psl, jc * JC : (jc + 1) * JC],
                        qh[b][psl, isl],
                        start=True, stop=True,
                    )
                es = work.tile([128, JP, IB], BF16, name="es")
                nc.scalar.activation(es[:], ps[:], AF.Exp)
                wt = work.tile([128, JP, IB], BF16, name="wt")
                meng = nc.gpsimd if (var == "pool" and b == 3) else nc.vector
                meng.tensor_mul(wt[:], es[:], ebt[:])
                if state["pending"] is not None:
                    emit_attnv(state["pending"])
                state["pending"] = (b, jg, wt)

        def emit_unit(ib, b, jg, ebt):
            isl = slice(ib * IB, (ib + 1) * IB)
            ps = psS.tile([128, JP, IB], F32, name="ps", tag="s")
            for t in range(JP):
                jc = jg * JP + t
                psl = slice(32 * t, 32 * (t + 1))
                nc.tensor.matmul(
                    ps[:, t, :],
                    kh[b][psl, jc * JC : (jc + 1) * JC],
                    qh[b][psl, isl],
                    start=True, stop=True,
                )
            es = work.tile([128, JP, IB], BF16, name="es")
            nc.scalar.activation(es[:], ps[:], AF.Exp)
            wt = work.tile([128, JP, IB], BF16, name="wt")
            nc.vector.tensor_mul(wt[:], es[:], ebt[:])
            if state["pending"] is not None:
                emit_attnv(state["pending"])
            state["pending"] = (b, jg, wt)

        def emit_attnv_half(h):
            b, jc, wt_t = h
            nc.tensor.matmul(
                po[b][:], vh[b][:, jc, :], wt_t[:],
                start=(jc == 0), stop=(jc == NJC - 1),
            )

        def emit_jg_esplit(ib, jg):
            isl = slice(ib * IB, (ib + 1) * IB)
            ebt = work.tile([128, JP, IB], BF16, name="ebt")
            nc.sync.dma_start(ebt[:], eb[ib, jg])
            for b in range(B):
                ps = psS.tile([128, JP, IB], F32, name="ps", tag="s")
                for t in range(JP):
                    jc = jg * JP + t
                    psl = slice(32 * t, 32 * (t + 1))
                    nc.tensor.matmul(
                        ps[:, t, :],
                        kh[b][psl, jc * JC : (jc + 1) * JC],
                        qh[b][psl, isl],
                        start=True, stop=True,
                    )
                for t in range(JP):
                    jc = jg * JP + t
                    es_t = work.tile([128, IB], BF16, name="esh")
                    nc.scalar.activation(es_t[:], ps[:, t, :], AF.Exp)
                    wt_t = work.tile([128, IB], BF16, name="wth")
                    nc.vector.tensor_mul(wt_t[:], es_t[:], ebt[:, t, :])
                    if state["hpend"]:
                        emit_attnv_half(state["hpend"].pop(0))
                    state["hpend"].append((b, jc, wt_t))

        def emit_tail_b(ib, b, po_b):
            ot = outp.tile([DH + 1, IB], F32, name="ot")
            nc.vector.tensor_copy(ot[:], po_b[:])
            zt = psO.tile([128, NIB], F32, name="zt", tag="po")
            for t_ in range(IB // JC):
                nc.tensor.matmul(
                    zt[:, t_ : t_ + 1],
                    ot[DH : DH + 1, t_ * JC : (t_ + 1) * JC],
                    ident[DH : DH + 1, :],
                    is_transpose=True,
                    skip_group_check=True,
                )
            rt = outp.tile([128, NIB], F32, name="rt")
            nc.vector.reciprocal(rt[:], zt[:])
            for t_ in range(IB // JC):
                pw = psO.tile([128, D], F32, name="pw", tag="po")
                nc.tensor.matmul(pw[:], ot[0:DH, t_ * JC : (t_ + 1) * JC], wo_sb[:], start=True, stop=True)
                ft = outp.tile([128, D], F32, name="ft")
                nc.vector.tensor_scalar_mul(ft[:], pw[:], rt[:, t_ : t_ + 1])
                r0 = ib * IB + t_ * JC
                nc.scalar.dma_start(out[b, r0 : r0 + JC, :], ft[:])

        if var == "bout":
            # batch-outer attention: one attn@v accumulator live at a time
            # (2 po banks double-buffered), 3 sim-psum slots, eb tiles cached
            # in SBUF across the 4 batch sweeps of each i-block
            for ib in range(NIB):
                ebts = [None] * NJG

                def get_ebt(jg, _ebts=ebts, _ib=ib):
                    if _ebts[jg] is None:
                        t = work.tile([128, JP, IB], BF16, name="ebt", tag="ebtc", bufs=16)
                        nc.sync.dma_start(t[:], eb[_ib, jg])
                        _ebts[jg] = t
                    return _ebts[jg]

                for b in range(B):
                    po_b = psO.tile([DH + 1, IB], F32, name="pob", tag="po")
                    po = [po_b] * B  # emit_attnv indexes po[b]
                    if ib == 0 and b == 0:
                        for ibk in range(NIB):
                            emit_proj_chunk(ibk)
                            emit_unit(0, 0, 2 * ibk, get_ebt(2 * ibk))
                            emit_unit(0, 0, 2 * ibk + 1, get_ebt(2 * ibk + 1))
                    else:
                        for jg in range(NJG):
                            emit_unit(ib, b, jg, get_ebt(jg))
                    emit_attnv(state["pending"])
                    state["pending"] = None
                    emit_tail_b(ib, b, po_b)
            return

        for ib in range(NIB):
            po = [psO.tile([DH + 1, IB], F32, name=f"po{b}", tag="po") for b in range(B)]
            if ib == 0:
                # interleave projection chunks with the first i-block's sweep:
                # attention(ib=0, jg) only needs k/v projection chunk jg//2
                ejg = emit_jg_esplit if var == "esplit" else emit_jg
                for ibk in range(NIB):
                    emit_proj_chunk(ibk)
                    ejg(0, 2 * ibk)
                    ejg(0, 2 * ibk + 1)
            else:
                for jg in range(NJG):
                    (emit_jg_esplit if var == "esplit" else emit_jg)(ib, jg)
            if state["pending"] is not None:
                emit_attnv(state["pending"])
            state["pending"] = None
            while state["hpend"]:
                emit_attnv_half(state["hpend"].pop(0))
            for b in range(B):
                ot = outp.tile([DH + 1, IB], F32, name="ot")
                if var == "otact":
                    nc.scalar.copy(ot[:], po[b][:])
                else:
                    nc.vector.tensor_copy(ot[:], po[b][:])
                zt = psO.tile([128, NIB], F32, name="zt", tag="po")
                for s in range(IB // JC):
                    nc.tensor.matmul(
                        zt[:, s : s + 1],
                        ot[DH : DH + 1, s * JC : (s + 1) * JC],
                        ident[DH : DH + 1, :],
                        is_transpose=True,
                        skip_group_check=True,
                    )
                rt = outp.tile([128, NIB], F32, name="rt")
                nc.vector.reciprocal(rt[:], zt[:])
                for s in range(IB // JC):
                    pw = psO.tile([128, D], F32, name="pw", tag="po")
                    nc.tensor.matmul(pw[:], ot[0:DH, s * JC : (s + 1) * JC], wo_sb[:], start=True, stop=True)
                    ft = outp.tile([128, D], F32, name="ft")
                    if var == "ftact" and s % 2 == 0:
                        nc.scalar.mul(ft[:], pw[:], rt[:, s : s + 1])
                    else:
                        nc.vector.tensor_scalar_mul(ft[:], pw[:], rt[:, s : s + 1])
                    r0 = ib * IB + s * JC
                    nc.sync.dma_start(out[b, r0 : r0 + JC, :], ft[:])


_CACHE = {}


def _get_nc(reps=1, var="base"):
    key = ("nc", reps, var)
    if key not in _CACHE:
        QKD = BF16 if QK_BF16 else F32
        nc = bacc.Bacc("TRN2", target_bir_lowering=False, debug=False, num_devices=NCORES)
        qT = nc.dram_tensor("qT", [B, NIB, 128, 2, IB], QKD, kind="ExternalInput")
        kT = nc.dram_tensor("kT", [B, NIB, 128, 2, IB], QKD, kind="ExternalInput")
        vT = nc.dram_tensor("vT", [B, NIB, 128, 2, IB], QKD, kind="ExternalInput")
        wq = nc.dram_tensor("wq", [128, 2, JP * DH], QKD, kind="ExternalInput")
        wk = nc.dram_tensor("wk", [128, 2, JP * DH], QKD, kind="ExternalInput")
        wv = nc.dram_tensor("wv", [128, 2, DH], QKD, kind="ExternalInput")
        wo = nc.dram_tensor("wo", [DH, D], F32, kind="ExternalInput")
        eb = nc.dram_tensor("eb", [NIB, NJG, 128, JP, IB], BF16, kind="ExternalInput")
        out = nc.dram_tensor("out", [B, N, D], F32, kind="ExternalOutput")
        build_kernel(
            nc,
            qT.ap(), kT.ap(), vT.ap(),
            wq.ap(), wk.ap(), wv.ap(), wo.ap(),
            eb.ap(), out.ap(),
            reps=reps, var=var,
        )
        nc.compile()
        _CACHE[key] = nc
    return _CACHE[key]


def _np_dtype():
    return ml_dtypes.bfloat16 if QK_BF16 else np.float32


def _dn_layout(x):
    """[B, N, D] -> [B, NIB, 128, 2, IB]; tile (b, ib)[p, c, col] = x[b, ib*IB+col, c*128+p]."""
    t = x.reshape(B, NIB, IB, 2, 128)
    return np.ascontiguousarray(t.transpose(0, 1, 4, 3, 2).astype(_np_dtype()))


def _w_layout(w, rep):
    """[32, 256] (out, in) -> [128, 2, rep*32] transposed, M-replicated."""
    wt = np.ascontiguousarray(w.T)                       # [256, 32]
    wt = np.concatenate([wt] * rep, axis=1)              # [256, rep*32]
    return np.ascontiguousarray(
        wt.reshape(2, 128, rep * DH).transpose(1, 0, 2).astype(_np_dtype())
    )


def _eb_layout(pb_h):
    """[N, N] pos_bias head -> [NIB, NJG, 128, JP, IB] tiled exp-bias (bf16).
    tile (ib, jg)[p, t, col] = exp(pb_h[ib*IB+col, jg*(JP*128)+t*128+p])."""
    e = np.exp(pb_h)                                     # [i, j]
    x = e.reshape(NIB, IB, NJG, JP, 128)                 # [ib, col, jg, t, p]
    return np.ascontiguousarray(x.transpose(0, 2, 4, 3, 1).astype(ml_dtypes.bfloat16))


def make_in_maps(q, k, v, pos_bias, Wq, Wk, Wv, Wo):
    q = np.asarray(q, dtype=np.float32)
    k = np.asarray(k, dtype=np.float32)
    v = np.asarray(v, dtype=np.float32)
    pos_bias = np.asarray(pos_bias, dtype=np.float32)
    Wq = np.asarray(Wq, dtype=np.float32)
    Wk = np.asarray(Wk, dtype=np.float32)
    Wv = np.asarray(Wv, dtype=np.float32)
    Wo = np.asarray(Wo, dtype=np.float32)

    qT = _dn_layout(q)
    kT = _dn_layout(k)
    vT = _dn_layout(v)

    in_maps = []
    for h in range(NCORES):
        hs = slice(h * DH, (h + 1) * DH)
        in_maps.append({
            "qT": qT,
            "kT": kT,
            "vT": vT,
            "wq": _w_layout(SCALE * Wq[hs, :], JP),
            "wk": _w_layout(Wk[hs, :], JP),
            "wv": _w_layout(Wv[hs, :], 1),
            "wo": np.ascontiguousarray(Wo[:, hs].T),
            "eb": _eb_layout(pos_bias[h]),
        })
    return in_maps


def kernel(q, k, v, pos_bias, Wq, Wk, Wv, Wo):
    nc = _get_nc()
    in_maps = make_in_maps(q, k, v, pos_bias, Wq, Wk, Wv, Wo)
    res = run_bass_kernel_spmd(nc, in_maps, core_ids=list(range(NCORES)))
    acc = res.results[0]["out"].astype(np.float32)
    for c in range(1, NCORES):
        acc = acc + res.results[c]["out"]
    return acc
